# revision 24
# baseline (speedup 1.0000x reference)
"""BinarizeLinear inference kernel for 8 Trainium2 NeuronCores.

Computes out = sign(input) @ sign(weight) + bias with sign(x) = +1 if x > 0
else -1, for input [8192, 4096] fp32, weight [4096, 4096] fp32, bias [4096].

Strategy: 4x2 (rows x cols) sharding across 8 cores. Each core computes a
[2048, 2048] output shard from x rows [2048, 4096] and w cols [4096, 2048].
On-chip per core:
  - binarize w and x to fp8e4 (+-1 is exact in fp8) with the ACT Sign LUT
  - transpose x tiles on the PE via a regular fp8 matmul against identity
    (xb_chunk.T @ I), copied PSUM->SBUF by the DVE
  - main GEMM in fp8 DoubleRow perf mode (256-deep contraction per matmul),
    accumulating exactly in fp32 PSUM (all partial sums are integers <= 4096)
  - bias add fused into the PSUM->SBUF copy on the DVE
"""

import numpy as np

M_FULL, K_FULL, N_FULL = 8192, 4096, 4096
R_SHARDS, C_SHARDS = 4, 2
N_CORES = R_SHARDS * C_SHARDS
M_SHARD = M_FULL // R_SHARDS  # 2048
N_SHARD = N_FULL // C_SHARDS  # 2048
P = 128
NT = 512  # moving free dim per matmul (one PSUM bank of fp32)


def build_nc(M=M_SHARD, K=K_FULL, N=N_SHARD, use_double_row=True, mblk_size=8):
    """Build the single-core Bass program (SPMD: same program on all cores).

    Loop structure (v2):
      - w is loaded n-block-major in [128, 4, 512] "quad" tiles (4 k-chunks)
        on the SP HWDGE queue, so the first output-block matmuls can start
        after ~1/NB of the w stream has landed.
      - x loads ride the ACT HWDGE queue and out stores the gpsimd SWDGE
        queue, so the three streams round-robin on the SDMA engines instead
        of serializing behind one another.
      - m-tiles are processed in blocks of `mblk_size`; within a block the
        output-column blocks (b) are the outer loop so PE stays dense while
        later w blocks stream in.
    """
    import concourse.mybir as mybir
    from concourse import bacc
    from concourse.masks import make_identity
    from concourse.tile import TileContext

    fp32 = mybir.dt.float32
    fp8 = mybir.dt.float8e4

    QUAD = 4  # k-chunks per w tile
    assert M % P == 0 and K % (P * QUAD) == 0 and N % NT == 0
    KSUB = K // P  # number of 128-deep k-chunks
    NQ = KSUB // QUAD  # w quad tiles per n-block
    NB = N // NT  # output column blocks
    MT = M // P  # m-tiles
    mblk_size = min(mblk_size, MT)
    assert MT % mblk_size == 0
    if use_double_row:
        assert KSUB % 2 == 0

    nc = bacc.Bacc()
    x = nc.declare_dram_parameter("x", [M, K], fp32, isOutput=False)
    # w is pre-permuted on the host into quad-major layout:
    # w_dev[b*NQ+q, ki, j, n] = w[(q*QUAD+j)*P + ki, b*NT + n], so each
    # [P, QUAD, NT] quad tile is one fully contiguous 1 MiB DMA.
    w = nc.declare_dram_parameter("w", [NB * NQ, P, QUAD, NT], fp32, isOutput=False)
    # bias comes pre-replicated across the 128 partitions from the host
    b = nc.declare_dram_parameter("b", [P, N], fp32, isOutput=False)
    out = nc.declare_dram_parameter("out", [M, N], fp32, isOutput=True)

    with TileContext(nc) as tc:
        with (
            tc.tile_pool(name="const", bufs=1) as cpool,
            tc.tile_pool(name="win", bufs=3) as winp,
            tc.tile_pool(name="wq", bufs=1) as wqp,
            tc.tile_pool(name="xin", bufs=2) as xinp,
            tc.tile_pool(name="xb", bufs=2) as xbp,
            tc.tile_pool(name="xbt", bufs=mblk_size + 2) as xbtp,
            tc.tile_pool(name="ost", bufs=4) as ostp,
            tc.tile_pool(name="tpsum", bufs=2, space="PSUM") as tpp,
            tc.tile_pool(name="mpsum", bufs=4, space="PSUM") as mpp,
            tc.tile_pool(name="wpsum", bufs=1, space="PSUM") as wpp,
        ):
            ident32 = cpool.tile([P, P], fp32)
            make_identity(nc, ident32)
            ident = cpool.tile([P, P], fp8)
            nc.vector.tensor_copy(ident, ident32)

            bias_rep = cpool.tile([P, N], fp32)
            nc.scalar.dma_start(bias_rep, b[:, :])

            # Binarized weight in n-block-major quad tiles: wq[b*NQ+q] holds
            # k-chunks 4q..4q+3 for output columns [b*NT, (b+1)*NT).
            wq = [None] * (NB * NQ)

            def emit_w_quad(bi, q):
                w_in = winp.tile(
                    [P, QUAD, NT], fp32, tag="w_in", name=f"w_in_{bi}_{q}"
                )
                nc.sync.dma_start(w_in, w[bi * NQ + q])
                wt = wqp.tile(
                    [P, QUAD, NT], fp8, tag=f"wq{bi}_{q}", name=f"wq_{bi}_{q}"
                )
                nc.scalar.sign(wt, w_in)
                wq[bi * NQ + q] = wt

            xbs = [None] * MT

            def emit_x(mi):
                x_in = xinp.tile([P, K], fp32, tag="x_in", name=f"x_in_{mi}")
                nc.sync.dma_start(x_in, x[mi * P : (mi + 1) * P, :])
                xb = xbp.tile([P, K], fp8, tag="xb", name=f"xb_{mi}")
                nc.scalar.sign(xb, x_in)
                xbs[mi] = xb

            # DMA / ACT issue order, matched to when the in-order PE needs
            # each piece: x0 first (first transposes), then all of w block 0
            # (gates the first matmul wave), x1, then x2..x7 interleaved with
            # w block 1, w block 2, x8-x9, w block 3, x10..x15.
            first_xs = list(range(min(mblk_size, MT)))
            if NB >= 2:
                emit_x(0)
                for q in range(NQ):
                    emit_w_quad(0, q)
                for mi in first_xs[1:2]:
                    emit_x(mi)
                rem = first_xs[2:]
                qi = 0
                for i in range(0, len(rem), 2):
                    for mi in rem[i : i + 2]:
                        emit_x(mi)
                    take = min(2, NQ - qi) if rem else NQ
                    for _ in range(take):
                        emit_w_quad(1, qi)
                        qi += 1
                while qi < NQ:
                    emit_w_quad(1, qi)
                    qi += 1
                for bi in range(2, NB):
                    for q in range(NQ):
                        emit_w_quad(bi, q)
                for mi in range(len(first_xs), MT):
                    emit_x(mi)
            else:
                for q in range(NQ):
                    emit_w_quad(0, q)
                for mi in range(MT):
                    emit_x(mi)

            # PE warmup: ~250 back-to-back small matmuls bridge the w-block-0
            # DMA wait and move the HAM clock gate to 2.4 GHz before the real
            # matmul stream starts.
            if NB >= 2:
                warm = wpp.tile([P, P], fp32, tag="warm", name="warm")
                for _ in range(150):
                    nc.tensor.matmul(warm, ident, ident, start=True, stop=True)

            def mm_group(mp, xbT, bi):
                if use_double_row:
                    for j2 in range(KSUB // 2):
                        q, r = divmod(j2, 2)
                        nc.tensor.matmul(
                            mp,
                            xbT[:, 2 * j2 : 2 * j2 + 2, :],
                            wq[bi * NQ + q][:, 2 * r : 2 * r + 2, :],
                            start=(j2 == 0),
                            stop=(j2 == KSUB // 2 - 1),
                            perf_mode=mybir.MatmulPerfMode.DoubleRow,
                        )
                else:
                    for j in range(KSUB):
                        q, r = divmod(j, QUAD)
                        nc.tensor.matmul(
                            mp,
                            xbT[:, j, :],
                            wq[bi * NQ + q][:, r, :],
                            start=(j == 0),
                            stop=(j == KSUB - 1),
                        )

            def emit_transpose(mi):
                # Transpose 128x128 chunks via PE matmul against identity,
                # four chunks per PSUM tile / DVE copy-back.
                xbT = xbtp.tile([P, KSUB, P], fp8, tag="xbT", name=f"xbT_{mi}")
                for g in range(KSUB // QUAD):
                    tp = tpp.tile([P, QUAD * P], fp32, tag="tp", name=f"tp_{mi}_{g}")
                    for t in range(QUAD):
                        j = g * QUAD + t
                        nc.tensor.matmul(
                            tp[:, t * P : (t + 1) * P],
                            xbs[mi][:, j * P : (j + 1) * P],
                            ident,
                            start=True,
                            stop=True,
                        )
                    nc.vector.tensor_copy(xbT[:, g * QUAD : (g + 1) * QUAD, :], tp)
                return xbT

            def emit_group(xbT, mi, bi, split=False):
                bsl = slice(bi * NT, (bi + 1) * NT)
                ost = ostp.tile([P, NT], fp32, tag="ost", name=f"ost_{mi}_{bi}")
                if split and use_double_row and KSUB % 4 == 0:
                    # Two half-depth accumulation groups so the first matmuls
                    # only gate on the first half of the w block's quads.
                    half = KSUB // 4
                    for h in range(2):
                        mp = mpp.tile(
                            [P, NT], fp32, tag="mp", name=f"mp_{mi}_{bi}_{h}"
                        )
                        for j2 in range(h * half, (h + 1) * half):
                            q, r = divmod(j2, 2)
                            nc.tensor.matmul(
                                mp,
                                xbT[:, 2 * j2 : 2 * j2 + 2, :],
                                wq[bi * NQ + q][:, 2 * r : 2 * r + 2, :],
                                start=(j2 == h * half),
                                stop=(j2 == (h + 1) * half - 1),
                                perf_mode=mybir.MatmulPerfMode.DoubleRow,
                            )
                        if h == 0:
                            nc.vector.tensor_copy(ost, mp)
                        else:
                            # both halves are exact integers; summing them
                            # first and adding bias last matches the
                            # reference's rounding exactly
                            nc.vector.tensor_tensor(
                                ost, mp, ost, op=mybir.AluOpType.add
                            )
                            nc.vector.tensor_tensor(
                                ost, ost, bias_rep[:, bsl], op=mybir.AluOpType.add
                            )
                else:
                    mp = mpp.tile([P, NT], fp32, tag="mp", name=f"mp_{mi}_{bi}")
                    mm_group(mp, xbT, bi)
                    nc.vector.tensor_tensor(
                        ost, mp, bias_rep[:, bsl], op=mybir.AluOpType.add
                    )
                nc.gpsimd.dma_start(out[mi * P : (mi + 1) * P, bsl], ost)

            # PE order: per m-block, transpose + first-block group per m-tile
            # (so PE starts as soon as x0 and w block 0 land), then the
            # remaining column-block waves.
            for mb in range(MT // mblk_size):
                blk = list(range(mb * mblk_size, (mb + 1) * mblk_size))
                xbts = {}
                for mi in blk:
                    xbts[mi] = emit_transpose(mi)
                    emit_group(xbts[mi], mi, 0)
                for bi in range(1, NB):
                    for mi in blk:
                        emit_group(xbts[mi], mi, bi)
    nc.finalize()
    return nc


def permute_w(w_col, K=K_FULL, N=N_SHARD, quad=4, nt=NT):
    """[K, N] -> [NB*NQ, P, QUAD, NT] quad-major device layout."""
    nq = K // (P * quad)
    nb = N // nt
    r = w_col.reshape(nq, quad, P, nb, nt)
    return np.ascontiguousarray(
        r.transpose(3, 0, 2, 1, 4).reshape(nb * nq, P, quad, nt)
    )


def _make_in_maps(input, weight, bias):
    x_np = np.asarray(input, dtype=np.float32)
    w_np = np.asarray(weight, dtype=np.float32)
    b_np = np.asarray(bias, dtype=np.float32).reshape(1, -1)
    w_cols = [
        permute_w(w_np[:, c * N_SHARD : (c + 1) * N_SHARD])
        for c in range(C_SHARDS)
    ]
    b_cols = [
        np.ascontiguousarray(
            np.broadcast_to(b_np[:, c * N_SHARD : (c + 1) * N_SHARD], (P, N_SHARD))
        )
        for c in range(C_SHARDS)
    ]
    in_maps = []
    for core in range(N_CORES):
        r, c = divmod(core, C_SHARDS)
        in_maps.append(
            {
                "x": np.ascontiguousarray(x_np[r * M_SHARD : (r + 1) * M_SHARD, :]),
                "w": w_cols[c],
                "b": b_cols[c],
            }
        )
    return in_maps


def _assemble(results):
    out = np.empty((M_FULL, N_FULL), dtype=np.float32)
    for core in range(N_CORES):
        r, c = divmod(core, C_SHARDS)
        out[r * M_SHARD : (r + 1) * M_SHARD, c * N_SHARD : (c + 1) * N_SHARD] = (
            results[core]["out"]
        )
    return out


def run(input, weight, bias, trace=False, trace_cores=None):
    """Run on 8 NeuronCores; returns (output, BassKernelResults)."""
    from concourse.bass_utils import run_bass_kernel_spmd

    nc = build_nc()
    in_maps = _make_in_maps(input, weight, bias)
    res = run_bass_kernel_spmd(
        nc, in_maps, list(range(N_CORES)), trace=trace, trace_cores=trace_cores
    )
    return _assemble(res.results), res


def kernel(input, weight, bias):
    out, _ = run(input, weight, bias)
    return out


# revision 25
# speedup vs baseline: 1.0456x; 1.0456x over previous
"""BinarizeLinear inference kernel for 8 Trainium2 NeuronCores.

Computes out = sign(input) @ sign(weight) + bias with sign(x) = +1 if x > 0
else -1, for input [8192, 4096] fp32, weight [4096, 4096] fp32, bias [4096].

Strategy: 4x2 (rows x cols) sharding across 8 cores. Each core computes a
[2048, 2048] output shard from x rows [2048, 4096] and w cols [4096, 2048].
On-chip per core:
  - binarize w and x to fp8e4 (+-1 is exact in fp8) with the ACT Sign LUT
  - transpose x tiles on the PE via a regular fp8 matmul against identity
    (xb_chunk.T @ I), copied PSUM->SBUF by the DVE
  - main GEMM in fp8 DoubleRow perf mode (256-deep contraction per matmul),
    accumulating exactly in fp32 PSUM (all partial sums are integers <= 4096)
  - bias add fused into the PSUM->SBUF copy on the DVE
"""

import numpy as np

M_FULL, K_FULL, N_FULL = 8192, 4096, 4096
R_SHARDS, C_SHARDS = 4, 2
N_CORES = R_SHARDS * C_SHARDS
M_SHARD = M_FULL // R_SHARDS  # 2048
N_SHARD = N_FULL // C_SHARDS  # 2048
P = 128
NT = 512  # moving free dim per matmul (one PSUM bank of fp32)


def build_nc(M=M_SHARD, K=K_FULL, N=N_SHARD, use_double_row=True, mblk_size=8):
    """Build the single-core Bass program (SPMD: same program on all cores).

    Loop structure (v2):
      - w is loaded n-block-major in [128, 4, 512] "quad" tiles (4 k-chunks)
        on the SP HWDGE queue, so the first output-block matmuls can start
        after ~1/NB of the w stream has landed.
      - x loads ride the ACT HWDGE queue and out stores the gpsimd SWDGE
        queue, so the three streams round-robin on the SDMA engines instead
        of serializing behind one another.
      - m-tiles are processed in blocks of `mblk_size`; within a block the
        output-column blocks (b) are the outer loop so PE stays dense while
        later w blocks stream in.
    """
    import concourse.mybir as mybir
    from concourse import bacc
    from concourse.masks import make_identity
    from concourse.tile import TileContext

    fp32 = mybir.dt.float32
    fp8 = mybir.dt.float8e4

    QUAD = 4  # k-chunks per w tile
    assert M % P == 0 and K % (P * QUAD) == 0 and N % NT == 0
    KSUB = K // P  # number of 128-deep k-chunks
    NQ = KSUB // QUAD  # w quad tiles per n-block
    NB = N // NT  # output column blocks
    MT = M // P  # m-tiles
    mblk_size = min(mblk_size, MT)
    assert MT % mblk_size == 0
    if use_double_row:
        assert KSUB % 2 == 0

    nc = bacc.Bacc()
    x = nc.declare_dram_parameter("x", [M, K], fp32, isOutput=False)
    # w is pre-permuted on the host into quad-major layout:
    # w_dev[b*NQ+q, ki, j, n] = w[(q*QUAD+j)*P + ki, b*NT + n], so each
    # [P, QUAD, NT] quad tile is one fully contiguous 1 MiB DMA.
    w = nc.declare_dram_parameter("w", [NB * NQ, P, QUAD, NT], fp32, isOutput=False)
    # bias comes pre-replicated across the 128 partitions from the host
    b = nc.declare_dram_parameter("b", [P, N], fp32, isOutput=False)
    out = nc.declare_dram_parameter("out", [M, N], fp32, isOutput=True)

    with TileContext(nc) as tc:
        with (
            tc.tile_pool(name="const", bufs=1) as cpool,
            tc.tile_pool(name="win", bufs=3) as winp,
            tc.tile_pool(name="wq", bufs=1) as wqp,
            tc.tile_pool(name="xin", bufs=2) as xinp,
            tc.tile_pool(name="xb", bufs=2) as xbp,
            tc.tile_pool(name="xbt", bufs=mblk_size + 2) as xbtp,
            tc.tile_pool(name="ost", bufs=4) as ostp,
            tc.tile_pool(name="tpsum", bufs=2, space="PSUM") as tpp,
            tc.tile_pool(name="mpsum", bufs=4, space="PSUM") as mpp,
            tc.tile_pool(name="wpsum", bufs=1, space="PSUM") as wpp,
        ):
            ident32 = cpool.tile([P, P], fp32)
            make_identity(nc, ident32)
            ident = cpool.tile([P, P], fp8)
            nc.vector.tensor_copy(ident, ident32)

            bias_rep = cpool.tile([P, N], fp32)
            nc.scalar.dma_start(bias_rep, b[:, :])

            # Binarized weight in n-block-major quad tiles: wq[b*NQ+q] holds
            # k-chunks 4q..4q+3 for output columns [b*NT, (b+1)*NT).
            wq = [None] * (NB * NQ)

            def emit_w_quad(bi, q):
                w_in = winp.tile(
                    [P, QUAD, NT], fp32, tag="w_in", name=f"w_in_{bi}_{q}"
                )
                nc.sync.dma_start(w_in, w[bi * NQ + q])
                wt = wqp.tile(
                    [P, QUAD, NT], fp8, tag=f"wq{bi}_{q}", name=f"wq_{bi}_{q}"
                )
                nc.scalar.sign(wt, w_in)
                wq[bi * NQ + q] = wt

            xbs = [None] * MT

            def emit_x(mi):
                x_in = xinp.tile([P, K], fp32, tag="x_in", name=f"x_in_{mi}")
                nc.sync.dma_start(x_in, x[mi * P : (mi + 1) * P, :])
                xb = xbp.tile([P, K], fp8, tag="xb", name=f"xb_{mi}")
                nc.scalar.sign(xb, x_in)
                xbs[mi] = xb

            # DMA / ACT issue order, matched to when the in-order PE needs
            # each piece: x0 first (first transposes), then all of w block 0
            # (gates the first matmul wave), x1, then x2..x7 interleaved with
            # w block 1, w block 2, x8-x9, w block 3, x10..x15.
            first_xs = list(range(min(mblk_size, MT)))
            if NB >= 2:
                emit_x(0)
                for q in range(NQ):
                    emit_w_quad(0, q)
                for mi in first_xs[1:2]:
                    emit_x(mi)
                rem = first_xs[2:]
                qi = 0
                for i in range(0, len(rem), 2):
                    for mi in rem[i : i + 2]:
                        emit_x(mi)
                    take = min(2, NQ - qi) if rem else NQ
                    for _ in range(take):
                        emit_w_quad(1, qi)
                        qi += 1
                while qi < NQ:
                    emit_w_quad(1, qi)
                    qi += 1
                for bi in range(2, NB):
                    for q in range(NQ):
                        emit_w_quad(bi, q)
                for mi in range(len(first_xs), MT):
                    emit_x(mi)
            else:
                for q in range(NQ):
                    emit_w_quad(0, q)
                for mi in range(MT):
                    emit_x(mi)

            # PE warmup: ~250 back-to-back small matmuls bridge the w-block-0
            # DMA wait and move the HAM clock gate to 2.4 GHz before the real
            # matmul stream starts.
            if NB >= 2:
                warm = wpp.tile([P, P], fp32, tag="warm", name="warm")
                for _ in range(150):
                    nc.tensor.matmul(warm, ident, ident, start=True, stop=True)

            def mm_group(mp, xbT, bi):
                if use_double_row:
                    for j2 in range(KSUB // 2):
                        q, r = divmod(j2, 2)
                        nc.tensor.matmul(
                            mp,
                            xbT[:, 2 * j2 : 2 * j2 + 2, :],
                            wq[bi * NQ + q][:, 2 * r : 2 * r + 2, :],
                            start=(j2 == 0),
                            stop=(j2 == KSUB // 2 - 1),
                            perf_mode=mybir.MatmulPerfMode.DoubleRow,
                        )
                else:
                    for j in range(KSUB):
                        q, r = divmod(j, QUAD)
                        nc.tensor.matmul(
                            mp,
                            xbT[:, j, :],
                            wq[bi * NQ + q][:, r, :],
                            start=(j == 0),
                            stop=(j == KSUB - 1),
                        )

            def emit_transpose(mi):
                # Transpose 128x128 chunks via PE matmul against identity,
                # four chunks per PSUM tile / DVE copy-back.
                xbT = xbtp.tile([P, KSUB, P], fp8, tag="xbT", name=f"xbT_{mi}")
                for g in range(KSUB // QUAD):
                    tp = tpp.tile([P, QUAD * P], fp32, tag="tp", name=f"tp_{mi}_{g}")
                    for t in range(QUAD):
                        j = g * QUAD + t
                        nc.tensor.matmul(
                            tp[:, t * P : (t + 1) * P],
                            xbs[mi][:, j * P : (j + 1) * P],
                            ident,
                            start=True,
                            stop=True,
                        )
                    nc.vector.tensor_copy(xbT[:, g * QUAD : (g + 1) * QUAD, :], tp)
                return xbT

            def emit_group(xbT, mi, bi):
                bsl = slice(bi * NT, (bi + 1) * NT)
                mp = mpp.tile([P, NT], fp32, tag="mp", name=f"mp_{mi}_{bi}")
                mm_group(mp, xbT, bi)
                ost = ostp.tile([P, NT], fp32, tag="ost", name=f"ost_{mi}_{bi}")
                nc.vector.tensor_tensor(
                    ost, mp, bias_rep[:, bsl], op=mybir.AluOpType.add
                )
                nc.gpsimd.dma_start(out[mi * P : (mi + 1) * P, bsl], ost)

            # PE order: per m-block, transpose + first-block group per m-tile
            # (so PE starts as soon as x0 and w block 0 land), then the
            # remaining column-block waves.
            for mb in range(MT // mblk_size):
                blk = list(range(mb * mblk_size, (mb + 1) * mblk_size))
                xbts = {}
                for mi in blk:
                    xbts[mi] = emit_transpose(mi)
                    emit_group(xbts[mi], mi, 0)
                for bi in range(1, NB):
                    for mi in blk:
                        emit_group(xbts[mi], mi, bi)
    nc.finalize()
    return nc


def permute_w(w_col, K=K_FULL, N=N_SHARD, quad=4, nt=NT):
    """[K, N] -> [NB*NQ, P, QUAD, NT] quad-major device layout."""
    nq = K // (P * quad)
    nb = N // nt
    r = w_col.reshape(nq, quad, P, nb, nt)
    return np.ascontiguousarray(
        r.transpose(3, 0, 2, 1, 4).reshape(nb * nq, P, quad, nt)
    )


def _make_in_maps(input, weight, bias):
    x_np = np.asarray(input, dtype=np.float32)
    w_np = np.asarray(weight, dtype=np.float32)
    b_np = np.asarray(bias, dtype=np.float32).reshape(1, -1)
    w_cols = [
        permute_w(w_np[:, c * N_SHARD : (c + 1) * N_SHARD])
        for c in range(C_SHARDS)
    ]
    b_cols = [
        np.ascontiguousarray(
            np.broadcast_to(b_np[:, c * N_SHARD : (c + 1) * N_SHARD], (P, N_SHARD))
        )
        for c in range(C_SHARDS)
    ]
    in_maps = []
    for core in range(N_CORES):
        r, c = divmod(core, C_SHARDS)
        in_maps.append(
            {
                "x": np.ascontiguousarray(x_np[r * M_SHARD : (r + 1) * M_SHARD, :]),
                "w": w_cols[c],
                "b": b_cols[c],
            }
        )
    return in_maps


def _assemble(results):
    out = np.empty((M_FULL, N_FULL), dtype=np.float32)
    for core in range(N_CORES):
        r, c = divmod(core, C_SHARDS)
        out[r * M_SHARD : (r + 1) * M_SHARD, c * N_SHARD : (c + 1) * N_SHARD] = (
            results[core]["out"]
        )
    return out


def run(input, weight, bias, trace=False, trace_cores=None):
    """Run on 8 NeuronCores; returns (output, BassKernelResults)."""
    from concourse.bass_utils import run_bass_kernel_spmd

    nc = build_nc()
    in_maps = _make_in_maps(input, weight, bias)
    res = run_bass_kernel_spmd(
        nc, in_maps, list(range(N_CORES)), trace=trace, trace_cores=trace_cores
    )
    return _assemble(res.results), res


def kernel(input, weight, bias):
    out, _ = run(input, weight, bias)
    return out
